# revision 8
# baseline (speedup 1.0000x reference)
"""LSTM layer with per-step weights on 8 trn2 NeuronCores — v2.

Tensor-parallel over the hidden dim (1024 -> 8 shards of 128 units/core),
with three structural changes vs the collective-AllGather baseline:

1. Matmul orientation: xh is the STATIONARY operand, weights STREAM.
   out[batch=128, 4*128 gate units] accumulates over 12 K-chunks of xh
   (4 from x, 8 from h); each chunk is one ldweights + one N=512 matmul.
   12 ldweights/step instead of 48, and the weight side pays no ld cost.
   Biases are folded in via a K=1 ones-row matmul at group start.

2. All state lives as [batch(partitions), units(free)]. The produced h
   shard is PE-transposed (tanh(c) and o transposed separately, then
   multiplied during the PSUM->SBUF cast) into [units, batch] fp8 — the
   exact lhsT layout the next step consumes.

3. The h exchange is the ncfw AllGather (fp8 payload, HBM bounce) as in
   the original baseline — the scheduler and runtime fully understand its
   synchronization, so no manual semaphores are needed.
"""

import numpy as np
import ml_dtypes

T, B, IN, H, O = 24, 128, 512, 1024, 256
NC = 8
SH = H // NC  # 128 hidden units per core
OS = O // NC  # 32 output channels per core
NKX = IN // 128  # 4 x chunks
NKH = H // 128  # 8 h chunks
NK = NKX + NKH  # 12 contraction chunks

BF16 = ml_dtypes.bfloat16
F8 = ml_dtypes.float8_e4m3

_cached = {}


def _build_module():
    import concourse.bacc as bacc
    import concourse.tile as tile
    import concourse.mybir as mybir

    f32 = mybir.dt.float32
    bf16 = mybir.dt.bfloat16
    f8 = mybir.dt.float8e4
    AF = mybir.ActivationFunctionType
    ALU = mybir.AluOpType

    nc = bacc.Bacc(
        "TRN2",
        target_bir_lowering=False,
        debug=False,
        enable_asserts=False,
        num_devices=NC,
    )

    xt_d = nc.dram_tensor("xt", [T, 128, IN], bf16, kind="ExternalInput")
    wt_d = nc.dram_tensor("wt", [T, 128, NK * 512], bf16, kind="ExternalInput")
    wo_d = nc.dram_tensor("wo", [128, T * NKH * OS], bf16, kind="ExternalInput")
    gb_d = nc.dram_tensor("gb", [1, T * 512], bf16, kind="ExternalInput")
    yb_d = nc.dram_tensor("yb", [1, T * OS], bf16, kind="ExternalInput")
    ones_d = nc.dram_tensor("ones", [1, 128], bf16, kind="ExternalInput")
    id_d = nc.dram_tensor("ident", [128, 128], f32, kind="ExternalInput")
    h0g_d = nc.dram_tensor("h0g", [128, NKH * 128], f8, kind="ExternalInput")
    c0_d = nc.dram_tensor("c0", [128, 128], f32, kind="ExternalInput")
    y_d = nc.dram_tensor("y", [128, T * OS], f32, kind="ExternalOutput")

    from concourse.tile_rust import add_dep_helper

    with tile.TileContext(nc) as tc:
        with (
            tc.tile_pool(name="const", bufs=1) as cpool,
            tc.tile_pool(name="w", bufs=4) as wpool,
            tc.tile_pool(name="g", bufs=2) as gpool,
            tc.tile_pool(name="tmp", bufs=2) as tpool,
            tc.tile_pool(name="psg", bufs=2, space="PSUM") as psgpool,
            tc.tile_pool(name="psy", bufs=2, space="PSUM") as psypool,
            tc.tile_pool(name="pst", bufs=4, space="PSUM") as pstpool,
            tc.tile_pool(name="dram", bufs=3, space="DRAM") as dpool,
        ):
            # resident tensors
            xt_sb = cpool.tile([128, T * IN], bf16)
            xv = xt_sb[:].rearrange("p (t n) -> p t n", t=T)
            xsrc = xt_d[:].rearrange("t p n -> p t n")
            for q in range(4):
                nc.scalar.dma_start(
                    xv[:, q * (T // 4) : (q + 1) * (T // 4), :],
                    xsrc[:, q * (T // 4) : (q + 1) * (T // 4), :],
                )
            wo_sb = cpool.tile([128, T * NKH * OS], bf16)
            gb_sb = cpool.tile([1, T * 512], bf16)
            nc.scalar.dma_start(gb_sb[:], gb_d[:])
            yb_sb = cpool.tile([1, T * OS], bf16)
            nc.scalar.dma_start(yb_sb[:], yb_d[:])
            ones_sb = cpool.tile([1, 128], bf16)
            nc.scalar.dma_start(ones_sb[:], ones_d[:])
            id_sb = cpool.tile([128, 128], f32)
            nc.sync.dma_start(id_sb[:], id_d[:])
            c_sb = cpool.tile([128, 128], f32)
            nc.scalar.dma_start(c_sb[:], c0_d[:])
            hg0 = cpool.tile([128, NKH * 128], f8)
            nc.sync.dma_start(hg0[:], h0g_d[:])
            hg1 = cpool.tile([128, NKH * 128], f8)
            y_all = cpool.tile([128, T * OS], f32)
            hg = [hg0, hg1]

            for t in range(T):
                if t == 1:
                    # deferred: wo is first consumed by step 1's y-matmuls,
                    # so it must not delay step 0's weight half on this ring
                    nc.sync.dma_start(wo_sb[:], wo_d[:])
                wt_sb = wpool.tile([128, NK * 512], bf16, name=f"wt{t}", tag="w")
                nc.sync.dma_start(wt_sb[:, 0 : 6 * 512], wt_d[t][:, 0 : 6 * 512])
                nc.scalar.dma_start(
                    wt_sb[:, 6 * 512 : NK * 512], wt_d[t][:, 6 * 512 : NK * 512]
                )

                # ---- PE stream ----
                # PSUM tiles are padded to full 2KB banks: a PE write and a
                # DVE/ACT read in the same bank are a fatal HW collision, so
                # no two live tiles may share a bank.
                psg = psgpool.tile([128, 512], f32, name=f"psg{t}", tag="psg")
                if t >= 1:
                    psyf = psypool.tile([128, 512], f32, name=f"psy{t}", tag="psy")
                    psy = psyf[:, 0:OS]
                    nc.tensor.matmul(
                        psy, ones_sb[:], yb_sb[:, (t - 1) * OS : t * OS],
                        start=True, stop=False,
                    )
                nc.tensor.matmul(
                    psg[:], ones_sb[:], gb_sb[:, t * 512 : (t + 1) * 512],
                    start=True, stop=False,
                )
                last_x_mm = None
                for kc in range(NKX):
                    col = (t * NKX + kc) * 128
                    last_x_mm = nc.tensor.matmul(
                        psg[:], xt_sb[:, col : col + 128],
                        wt_sb[:, kc * 512 : (kc + 1) * 512],
                        start=False, stop=False,
                    )
                hgb = hg[t % 2]
                for s in range(NKH):
                    nc.tensor.matmul(
                        psg[:], hgb[:, s * 128 : (s + 1) * 128],
                        wt_sb[:, (NKX + s) * 512 : (NKX + s + 1) * 512],
                        start=False, stop=(s == NKH - 1),
                    )
                last_y_mm = None
                if t >= 1:
                    for s in range(NKH):
                        col = ((t - 1) * NKH + s) * OS
                        last_y_mm = nc.tensor.matmul(
                            psy, hgb[:, s * 128 : (s + 1) * 128],
                            wo_sb[:, col : col + OS],
                            start=False, stop=(s == NKH - 1),
                        )

                # ---- gate activations ----
                zt = gpool.tile([128, 128], f32, name=f"zt{t}", tag="zt")
                nc.scalar.activation(zt[:], psg[:, 0:128], AF.Tanh)
                iot = gpool.tile([128, 384], f32, name=f"iot{t}", tag="iot")
                nc.scalar.activation(iot[:], psg[:, 128:512], AF.Sigmoid)
                # transpose o on PE (off the serial chain)
                pstof = pstpool.tile([128, 512], f32, name=f"psto{t}", tag="pst")
                psto = pstof[:, 0:128]
                nc.tensor.transpose(psto, iot[:, 256:384], id_sb[:])
                # PSUM has a single DVE read port: stage o^T to SBUF so the
                # h^T combine reads only one PSUM operand.
                oT = gpool.tile([128, 128], f32, name=f"oT{t}", tag="oT")
                nc.scalar.copy(oT[:], psto)

                # ---- c/h update ----
                t1 = tpool.tile([128, 128], f32, name=f"t1_{t}", tag="t1")
                nc.vector.scalar_tensor_tensor(
                    t1[:], zt[:], 0.0, iot[:, 0:128], ALU.bypass, ALU.mult
                )
                # t2 on DVE, not gpsimd: gpsimd ucode tensor ops would force
                # a per-step library reload war with the remote-dma library.
                t2 = tpool.tile([128, 128], f32, name=f"t2_{t}", tag="t2")
                nc.vector.scalar_tensor_tensor(
                    t2[:], c_sb[:], 0.0, iot[:, 128:256], ALU.bypass, ALU.mult
                )
                cadd = nc.vector.scalar_tensor_tensor(
                    c_sb[:], t1[:], 0.0, t2[:], ALU.bypass, ALU.add
                )
                tct = tpool.tile([128, 128], f32, name=f"tc{t}", tag="tc")
                nc.scalar.activation(tct[:], c_sb[:], AF.Tanh)
                pstcf = pstpool.tile([128, 512], f32, name=f"pstc{t}", tag="pst")
                pstc = pstcf[:, 0:128]
                nc.tensor.transpose(pstc, tct[:], id_sb[:])
                hgn = hg[(t + 1) % 2]
                # h^T = tanh(c)^T * o^T, cast to fp8, into the send staging
                hstage = tpool.tile([128, 128], f8, name=f"hst{t}", tag="hst")
                nc.vector.scalar_tensor_tensor(
                    hstage[:], pstc, 0.0, oT[:], ALU.bypass, ALU.mult
                )

                # y(t-1) activation (off critical path)
                if t >= 1:
                    nc.scalar.activation(
                        y_all[:, (t - 1) * OS : t * OS], psy, AF.Sigmoid
                    )

                # ---- exchange: ncfw AllGather via HBM bounce ----
                hsh = dpool.tile([128, B], f8, tag="hsh", name=f"hsh{t}")
                nc.sync.dma_start(hsh[:], hstage[:])
                hgd = dpool.tile(
                    [NC * 128, B], f8, tag="hgd", name=f"hgd{t}",
                    addr_space="Shared",
                )
                nc.gpsimd.collective_compute(
                    "AllGather",
                    ALU.bypass,
                    replica_groups=[list(range(NC))],
                    ins=[hsh.opt()],
                    outs=[hgd.opt()],
                )
                hv = hgn[:].rearrange("p (s b) -> p s b", s=NC)
                gv = hgd[:].rearrange("(s p) b -> p s b", p=128)
                nc.scalar.dma_start(hv[:, 0:4, :], gv[:, 0:4, :])
                nc.sync.dma_start(hv[:, 4:8, :], gv[:, 4:8, :])

            # ---- epilogue: y(T-1) from the final gather ----
            psyf = psypool.tile([128, 512], f32, name="psyF", tag="psy")
            psy = psyf[:, 0:OS]
            nc.tensor.matmul(
                psy, ones_sb[:], yb_sb[:, (T - 1) * OS : T * OS],
                start=True, stop=False,
            )
            hgb = hg[T % 2]
            for s in range(NKH):
                col = ((T - 1) * NKH + s) * OS
                nc.tensor.matmul(
                    psy, hgb[:, s * 128 : (s + 1) * 128],
                    wo_sb[:, col : col + OS],
                    start=False, stop=(s == NKH - 1),
                )
            nc.scalar.activation(y_all[:, (T - 1) * OS : T * OS], psy, AF.Sigmoid)
            nc.sync.dma_start(y_d[:], y_all[:])

    nc.compile()
    return nc


def _prep_inputs(x, W, Wi, Wf, Wo, Wout, b, bi, bf, bo, bout, c0, h0):
    """Build the 8 per-core input maps (host-side layout shuffling)."""
    # xt[t, p, kc*128+bb] = x[bb, t, kc*128+p]  (x^T chunks = lhsT layout;
    # shared by all cores)
    xt = (
        x.transpose(1, 2, 0)
        .reshape(T, NKX, 128, B)
        .transpose(0, 2, 1, 3)
        .reshape(T, 128, IN)
    )
    xt = np.ascontiguousarray(xt).astype(BF16)

    Wall = np.stack([W, Wi, Wf, Wo], axis=1)  # [T, 4, H, IN+H]
    Ball = np.stack([b, bi, bf, bo], axis=1)  # [T, 4, H]

    ones = np.ones((1, 128), dtype=BF16)
    ident = np.eye(128, dtype=np.float32)

    in_maps = []
    for c in range(NC):
        sh = slice(c * SH, (c + 1) * SH)
        osl = slice(c * OS, (c + 1) * OS)
        xor_perm = list(range(NKH))

        Wc = Wall[:, :, sh, :]  # [T, 4, 128, 1536]
        # x part: wt[t, p, j*512 + g*128 + m] = Wc[t, g, m, j*128+p]
        Wx = Wc[..., :IN].reshape(T, 4, SH, NKX, 128)  # t g m j p
        Xarr = Wx.transpose(0, 4, 3, 1, 2)  # t p j g m
        # h part, slot s holds chunk c^s
        Wh = Wc[..., IN:].reshape(T, 4, SH, NKH, 128)  # t g m k p
        Wh = Wh[:, :, :, xor_perm, :]  # k axis -> slot axis
        Harr = Wh.transpose(0, 4, 3, 1, 2)  # t p s g m
        wt = np.concatenate(
            [Xarr.reshape(T, 128, NKX * 512), Harr.reshape(T, 128, NKH * 512)],
            axis=2,
        )
        wt = np.ascontiguousarray(wt).astype(BF16)

        # wo[p, t*256 + s*32 + m] = Wout[t, c*32+m, (c^s)*128+p]
        Woc = Wout[:, osl, :].reshape(T, OS, NKH, 128)  # t m k p
        Woc = Woc[:, :, xor_perm, :]  # t m s p
        wo = np.ascontiguousarray(
            Woc.transpose(3, 0, 2, 1).reshape(128, T * NKH * OS)
        ).astype(BF16)

        gbias = np.ascontiguousarray(
            Ball[:, :, sh].reshape(1, T * 512)
        ).astype(BF16)
        ybias = np.ascontiguousarray(bout[:, osl].reshape(1, T * OS)).astype(BF16)

        # h0 gather: slot s = chunk c^s, value broadcast over batch cols
        h0r = h0.reshape(NKH, 128)
        h0g = np.empty((128, NKH * 128), dtype=F8)
        for s in range(NKH):
            h0g[:, s * 128 : (s + 1) * 128] = h0r[s][:, None].astype(F8)

        c0b = np.ascontiguousarray(
            np.broadcast_to(c0[sh][None, :], (128, 128))
        ).astype(np.float32)

        in_maps.append(
            {
                "xt": xt,
                "wt": wt,
                "wo": wo,
                "gb": gbias,
                "yb": ybias,
                "ones": ones,
                "ident": ident,
                "h0g": h0g,
                "c0": c0b,
            }
        )
    return in_maps


def kernel(**inputs):
    from concourse.bass_utils import run_bass_kernel_spmd

    inputs = {k: np.asarray(v, dtype=np.float32) for k, v in inputs.items()}
    in_maps = _prep_inputs(**inputs)

    if "nc" not in _cached:
        _cached["nc"] = _build_module()
    nc = _cached["nc"]

    res = run_bass_kernel_spmd(nc, in_maps, core_ids=list(range(NC)))
    ys = [r["y"].reshape(B, T, OS) for r in res.results]  # each [B, T, 32]
    Y = np.stack(ys, axis=2)  # [B, T, 8, 32]
    out = Y.reshape(B, T * O)
    return np.ascontiguousarray(out).astype(np.float32)


if __name__ == "__main__":
    rng = np.random.default_rng(0)
    ih = IN + H
    ins = {
        "x": rng.standard_normal((B, T, IN), dtype=np.float32),
        "W": rng.standard_normal((T, H, ih), dtype=np.float32) * 0.02,
        "Wi": rng.standard_normal((T, H, ih), dtype=np.float32) * 0.02,
        "Wf": rng.standard_normal((T, H, ih), dtype=np.float32) * 0.02,
        "Wo": rng.standard_normal((T, H, ih), dtype=np.float32) * 0.02,
        "Wout": rng.standard_normal((T, O, H), dtype=np.float32) * 0.02,
        "b": rng.standard_normal((T, H), dtype=np.float32) * 0.02,
        "bi": rng.standard_normal((T, H), dtype=np.float32) * 0.02,
        "bf": rng.standard_normal((T, H), dtype=np.float32) * 0.02,
        "bo": rng.standard_normal((T, H), dtype=np.float32) * 0.02,
        "bout": rng.standard_normal((T, O), dtype=np.float32) * 0.02,
        "c0": rng.standard_normal((H,), dtype=np.float32) * 0.02,
        "h0": rng.standard_normal((H,), dtype=np.float32) * 0.02,
    }
    out = kernel(**ins)
    print("kernel output", out.shape, out.dtype, float(np.abs(out).mean()))
